# revision 20
# baseline (speedup 1.0000x reference)
import os
import sys

sys.path.insert(0, "/opt/trn_rl_repo")

import math

import numpy as np

import concourse.bass as bass  # noqa: F401
import concourse.tile as tile
from concourse import bacc, bass_isa, mybir
from concourse.bass_utils import run_bass_kernel_spmd
from concourse.masks import make_identity

N_CORES = 8
B, C_IN, C_OUT, H, W, NATTR = 256, 2048, 512, 14, 14, 8
HW = H * W            # 196
BL = int(os.environ.get("K_BL", str(B // N_CORES)))  # batches per core (32)
STAGE = int(os.environ.get("K_STAGE", "9"))
NBC = 2               # batches per chunk
NCH = BL // NBC       # chunks per core (16)
NCOLS = NBC * HW      # 392
HWP = 256             # hw padded to >=256 for full-rate f32r matmuls
KT = C_IN // 128      # 16
MT = C_OUT // 128     # 4
EPS = 1e-5
RSQ512 = 1.0 / math.sqrt(512.0)
HWA = 128             # first hw tile rows
HWB = HW - HWA        # 68

F32 = mybir.dt.float32
F32R = mybir.dt.float32r
BF16 = mybir.dt.bfloat16
FP16 = mybir.dt.float16
I32 = mybir.dt.int32
Alu = mybir.AluOpType
Act = mybir.ActivationFunctionType
AxX = mybir.AxisListType.X

VECS = ["conv1_b", "bn_g", "bn_b", "bn_m", "bn_v", "fc1_b", "fc2_b"]
IV = {n: i for i, n in enumerate(VECS)}


def build_nc():
    nc = bacc.Bacc("TRN2", target_bir_lowering=False, debug=False, num_devices=N_CORES)

    dp = nc.declare_dram_parameter
    feat_e = dp("feat", [BL, C_IN, HW], F32, isOutput=False)
    c_e = dp("c", [BL], I32, isOutput=False)
    conv1wT_e = dp("conv1wT", [C_IN, C_OUT], F32, isOutput=False)
    attr_embT_e = dp("attr_embT", [512, NATTR], F32R, isOutput=False)
    t1wT_e = dp("t1wT", [512, 512], F32R, isOutput=False)
    t2wT_e = dp("t2wT", [512, 512], F32R, isOutput=False)
    fc1wT_e = dp("fc1wT", [1024, 512], F32R, isOutput=False)
    fc2wT_e = dp("fc2wT", [512, 512], F32R, isOutput=False)
    t1b_e = dp("t1_b", [512], F32R, isOutput=False)
    t2b_e = dp("t2_b", [512], F32R, isOutput=False)
    vcols_e = dp("vcols", [128, len(VECS), MT], F32, isOutput=False)
    masks_e = dp("masks_w", [NATTR, 512], F32R, isOutput=False)
    sa_e = dp("sa_toep", [2, HW, HW], F32, isOutput=False)
    w_e = dp("w", [1], F32, isOutput=False)
    w1_e = dp("w1", [1], F32, isOutput=False)
    w2_e = dp("w2", [1], F32, isOutput=False)
    out_e = dp("out", [BL, C_OUT], F32, isOutput=True)

    with tile.TileContext(nc) as tc:
        with tc.tile_pool(name="singles", bufs=1) as sg, \
             tc.tile_pool(name="fstg", bufs=2) as fstg, \
             tc.tile_pool(name="featp", bufs=3) as featp, \
             tc.tile_pool(name="xp", bufs=3) as xp, \
             tc.tile_pool(name="imgp", bufs=2) as imgp, \
             tc.tile_pool(name="chsm", bufs=2) as chsm, \
             tc.tile_pool(name="chbig", bufs=2) as chbig, \
             tc.tile_pool(name="pconv", bufs=3, space="PSUM") as pconv, \
             tc.tile_pool(name="psp", bufs=5, space="PSUM") as psp:

            # ---------------- phase 0: weights / tables ----------------
            # feat prefetch for chunks 0/1 goes first so the conv pipeline
            # starts as early as possible; weight DMAs follow.
            feat_tiles = {}

            def emit_feat(g):
                featbf = featp.tile([128, KT, NBC, HW], FP16, tag="feat", name="featbf")
                for b in range(NBC):
                    stg = fstg.tile([128, KT, HW], F32, tag="fstg", name="stg")
                    nc.sync.dma_start(
                        out=stg,
                        in_=feat_e[g * NBC + b].rearrange("(kt p) s -> p kt s", p=128))
                    nc.scalar.copy(out=featbf[:, 0:6, b, :], in_=stg[:, 0:6, :])
                    nc.vector.tensor_copy(out=featbf[:, 6:12, b, :], in_=stg[:, 6:12, :])
                    nc.gpsimd.tensor_copy(out=featbf[:, 12:16, b, :], in_=stg[:, 12:16, :])
                feat_tiles[g] = featbf

            for g0 in range(min(2, NCH)):
                emit_feat(g0)

            ident = sg.tile([128, 128], F32)
            make_identity(nc, ident[:])

            # conv weights: f32 staged -> fp16
            wT = sg.tile([128, KT, C_OUT], FP16)
            for q in range(4):
                wst = fstg.tile([128, 4, C_OUT], F32, tag="fstg")
                nc.sync.dma_start(
                    out=wst,
                    in_=conv1wT_e[q * 512:(q + 1) * 512].rearrange("(kt p) o -> p kt o", p=128))
                nc.vector.tensor_copy(out=wT[:, q * 4:(q + 1) * 4, :], in_=wst)

            attr_T = sg.tile([128, 4, NATTR], F32R)
            nc.sync.dma_start(out=attr_T, in_=attr_embT_e[:].rearrange("(t p) j -> p t j", p=128))
            t1wT = sg.tile([128, 4, 512], F32R)
            nc.sync.dma_start(out=t1wT, in_=t1wT_e[:].rearrange("(t p) i -> p t i", p=128))
            t1b_row = sg.tile([1, 512], F32R)
            nc.sync.dma_start(out=t1b_row, in_=t1b_e[:].unsqueeze(0))
            vcols = sg.tile([128, len(VECS), MT], F32)
            nc.sync.dma_start(out=vcols, in_=vcols_e[:])

            t2wT = sg.tile([128, 4, 512], F32R)
            fc1wT = sg.tile([128, 8, 512], F32R)
            fc2wT = sg.tile([128, 4, 512], F32R)
            t2b_row = sg.tile([1, 512], F32R)
            masks_sb = sg.tile([NATTR, 512], F32R)

            def emit_tail_weights():
                nc.sync.dma_start(out=t2wT, in_=t2wT_e[:].rearrange("(t p) i -> p t i", p=128))
                nc.sync.dma_start(out=fc1wT, in_=fc1wT_e[:].rearrange("(t p) i -> p t i", p=128))
                nc.sync.dma_start(out=fc2wT, in_=fc2wT_e[:].rearrange("(t p) i -> p t i", p=128))
                nc.sync.dma_start(out=t2b_row, in_=t2b_e[:].unsqueeze(0))
                nc.sync.dma_start(out=masks_sb, in_=masks_e[:])

            saWa = sg.tile([HWA, 2, HW], BF16)
            saWb = sg.tile([HWB, 2, HW], BF16)

            def emit_sa_weights():
                saWf_a = sg.tile([HWA, 2, HW], F32, name="saWf_a")
                nc.sync.dma_start(out=saWf_a, in_=sa_e[:, 0:HWA, :].rearrange("c p s -> p c s"))
                saWf_b = sg.tile([HWB, 2, HW], F32, name="saWf_b")
                nc.sync.dma_start(out=saWf_b, in_=sa_e[:, HWA:HW, :].rearrange("c p s -> p c s"))
                nc.vector.tensor_copy(out=saWa, in_=saWf_a)
                nc.vector.tensor_copy(out=saWb, in_=saWf_b)

            def load_scalar_bc(ext, name):
                s = sg.tile([1, 1], F32, tag=f"s_{name}")
                nc.sync.dma_start(out=s, in_=ext[:].unsqueeze(0))
                bc = sg.tile([128, 1], F32, tag=f"bc_{name}")
                nc.gpsimd.partition_broadcast(out_ap=bc[:], in_ap=s[:])
                return bc

            w0_bc = load_scalar_bc(w_e, "w0")
            w1_bc = load_scalar_bc(w1_e, "w1")
            w2_bc = load_scalar_bc(w2_e, "w2")

            # bn fold into tanh scale/bias
            epst = sg.tile([128, 1], F32)
            nc.vector.memset(epst[:], EPS)
            sqv = sg.tile([128, MT], F32)
            nc.scalar.activation(out=sqv[:], in_=vcols[:, IV["bn_v"], :], func=Act.Sqrt, bias=epst[:])
            rsv = sg.tile([128, MT], F32)
            nc.vector.reciprocal(out=rsv[:], in_=sqv[:])
            s0 = sg.tile([128, MT], F32)
            nc.vector.tensor_tensor(out=s0[:], in0=vcols[:, IV["bn_g"], :], in1=rsv[:], op=Alu.mult)
            act_scale = sg.tile([128, MT], F32)
            nc.vector.tensor_scalar(out=act_scale[:], in0=s0[:], scalar1=w0_bc[:], scalar2=None, op0=Alu.mult)
            tmpv = sg.tile([128, MT], F32)
            nc.vector.tensor_tensor(out=tmpv[:], in0=vcols[:, IV["conv1_b"], :], in1=vcols[:, IV["bn_m"], :], op=Alu.subtract)
            nc.vector.tensor_tensor(out=tmpv[:], in0=tmpv[:], in1=s0[:], op=Alu.mult)
            act_bias = sg.tile([128, MT], F32)
            nc.vector.tensor_tensor(out=act_bias[:], in0=tmpv[:], in1=vcols[:, IV["bn_b"], :], op=Alu.add)

            # onehot [8, BL] from c
            c_sb = sg.tile([1, BL], I32)
            nc.sync.dma_start(out=c_sb, in_=c_e[:].unsqueeze(0))
            c_f = sg.tile([1, BL], F32)
            nc.vector.tensor_copy(out=c_f, in_=c_sb)
            c_bc = sg.tile([NATTR, BL], F32)
            nc.gpsimd.partition_broadcast(out_ap=c_bc[:], in_ap=c_f[:])
            iota_i = sg.tile([NATTR, 1], I32)
            nc.gpsimd.iota(iota_i[:], pattern=[[0, 1]], base=0, channel_multiplier=1)
            iota_f = sg.tile([NATTR, 1], F32)
            nc.vector.tensor_copy(out=iota_f, in_=iota_i)
            onehot = sg.tile([NATTR, BL], F32R)
            nc.vector.tensor_scalar(out=onehot[:], in0=c_bc[:], scalar1=iota_f[:], scalar2=None, op0=Alu.is_equal)

            # ones / selector constants
            ones8 = sg.tile([1, NATTR], F32)
            nc.vector.memset(ones8[:], 1.0)
            ones8_r = sg.tile([1, NATTR], F32R)
            nc.vector.tensor_copy(out=ones8_r, in_=ones8)
            avg_ones = sg.tile([128, 1], F32)
            nc.vector.memset(avg_ones[:], 1.0 / C_OUT)
            avg_ones_r = sg.tile([128, 1], F32R)
            nc.vector.tensor_copy(out=avg_ones_r, in_=avg_ones)
            one_col = sg.tile([128, 1], F32)
            nc.vector.memset(one_col[:], 1.0)
            one_col_r = sg.tile([128, 1], F32R)
            nc.vector.tensor_copy(out=one_col_r, in_=one_col)

            sel1_f = sg.tile([2, 2, 128], F32)
            nc.gpsimd.memset(sel1_f[:], 0.0)
            nc.gpsimd.affine_select(
                out=sel1_f[:], in_=sel1_f[:], compare_op=Alu.not_equal, fill=1.0,
                base=0, pattern=[[-1, 2], [0, 128]], channel_multiplier=1,
            )
            sel1_r = sg.tile([2, 2, 128], F32R)
            nc.vector.tensor_copy(out=sel1_r, in_=sel1_f)
            ones_row = sg.tile([1, 128], F32)
            nc.vector.memset(ones_row[:], 1.0)
            w1_row = sg.tile([1, 128], F32R)
            nc.vector.tensor_scalar(out=w1_row[:], in0=ones_row[:], scalar1=w1_bc[0:1, 0:1], scalar2=None, op0=Alu.mult)

            def attn_table(twT, tb_row, scale, dst_tag):
                pat = psp.tile([NATTR, 512], F32, tag="sp")
                for t in range(4):
                    nc.tensor.matmul(pat[:], attr_T[:, t, :], twT[:, t, :], start=(t == 0), stop=False)
                nc.tensor.matmul(pat[:], ones8_r[:], tb_row[:], start=False, stop=True)
                tt_t = sg.tile([NATTR, 512], F32, tag=dst_tag + "_tanh")
                nc.scalar.activation(out=tt_t[:], in_=pat[:], func=Act.Tanh, scale=0.5)
                sig_t = sg.tile([NATTR, 512], F32, tag=dst_tag + "_sig")
                nc.vector.tensor_scalar(out=sig_t[:], in0=tt_t[:], scalar1=0.5, scalar2=0.5,
                                        op0=Alu.mult, op1=Alu.add)
                tab = sg.tile([NATTR, 512], F32R, tag=dst_tag)
                nc.vector.tensor_tensor(out=tab[:], in0=sig_t[:], in1=pat[:], op=Alu.mult)
                g = sg.tile([128, MT, BL], F32R, tag=dst_tag + "_g")
                for mt in range(MT):
                    gps = psp.tile([128, BL], F32, tag="sp")
                    nc.tensor.matmul(gps[:], tab[:, mt * 128:(mt + 1) * 128], onehot[:], start=True, stop=True)
                    nc.scalar.activation(out=g[:, mt, :], in_=gps[:], func=Act.Copy, scale=scale)
                return g

            aT_g = attn_table(t1wT, t1b_row, RSQ512, "a1")

            masksT = sg.tile([128, MT, BL], F32)

            def emit_tail_tables():
                g2 = attn_table(t2wT, t2b_row, 1.0, "a2")
                for mt in range(MT):
                    gps = psp.tile([128, BL], F32, tag="sp", name="gps")
                    nc.tensor.matmul(gps[:], masks_sb[:, mt * 128:(mt + 1) * 128], onehot[:], start=True, stop=True)
                    nc.scalar.activation(out=masksT[:, mt, :], in_=gps[:], func=Act.Relu)
                return g2

            xsumT = sg.tile([128, MT, BL], F32)

            # ---------------- software-pipelined chunk loop ----------------
            # Each chunk's epilogue is split into stages and emitted between the
            # conv m-tiles of later chunks so the PE instruction stream never
            # stalls on the DVE/ACT chain (keeps the HAM clock-gate warm).

            def emit_dma(st):
                g = st["g"]
                st["feat"] = feat_tiles.pop(g)
                st["x"] = xp.tile([128, MT, NBC, HW], F32R, tag="x", name="x")
                st["img"] = imgp.tile([128, MT, NBC, HWP], F32R, tag="img", name="img")

            def emit_conv_mt(st, mt):
                cps = pconv.tile([128, NCOLS], F32, tag="conv")
                for kt in range(KT):
                    nc.tensor.matmul(
                        cps[:], wT[:, kt, mt * 128:(mt + 1) * 128], st["feat"][:, kt],
                        start=(kt == 0), stop=(kt == KT - 1))
                # x = w0*conv + conv1_b (DVE) ; img = tanh(conv*s + b) (ACT)
                nc.scalar.activation(
                    out=st["x"][:, mt], in_=cps[:].rearrange("p (b s) -> p b s", b=NBC),
                    func=Act.Identity, bias=vcols[:, IV["conv1_b"], mt:mt + 1], scale=w0_bc[:])
                nc.scalar.activation(
                    out=st["img"][:, mt, :, 0:HW], in_=cps[:].rearrange("p (b s) -> p b s", b=NBC),
                    func=Act.Tanh, bias=act_bias[:, mt:mt + 1], scale=act_scale[:, mt:mt + 1])

            def epi_A(st):  # attmap matmuls + softmax
                g = st["g"]
                st["att_nrm"] = []
                for b in range(NBC):
                    aps = psp.tile([1, HWP], F32, tag="sp")
                    col = g * NBC + b
                    for mt in range(MT):
                        nc.tensor.matmul(aps[:], aT_g[:, mt, col:col + 1],
                                         st["img"][:, mt, b, :], start=(mt == 0), stop=(mt == MT - 1))
                    negmax = chsm.tile([1, 1], F32, tag="negmax")
                    nc.vector.tensor_reduce(out=negmax[:], in_=aps[0:1, 0:HW], axis=AxX,
                                            op=Alu.max, negate=True)
                    att_exp = chsm.tile([1, HW], F32, tag="att_exp")
                    sume = chsm.tile([1, 1], F32, tag="sume")
                    nc.scalar.activation(out=att_exp[:], in_=aps[0:1, 0:HW], func=Act.Exp,
                                         bias=negmax[:], scale=1.0, accum_out=sume[:])
                    rcp = chsm.tile([1, 1], F32, tag="rcp")
                    nc.vector.reciprocal(out=rcp[:], in_=sume[:])
                    att_nrm = chsm.tile([1, HWP], F32R, tag="att_nrm")
                    nc.vector.tensor_scalar(out=att_nrm[0:1, 0:HW], in0=att_exp[:], scalar1=rcp[:],
                                            scalar2=None, op0=Alu.mult)
                    st["att_nrm"].append(att_nrm)

            def epi_B(st):  # attmap broadcast, x *= w1*attmap, max combine + all-reduce
                bps = psp.tile([128, NBC, HWP], F32, tag="sp")
                for b in range(NBC):
                    nc.tensor.matmul(bps[:, b, :], w1_row[:], st["att_nrm"][b][:],
                                     start=True, stop=True)
                x = st["x"]
                for mt in range(MT):
                    nc.vector.tensor_tensor(out=x[:, mt], in0=x[:, mt].bitcast(F32),
                                            in1=bps[:, :, 0:HW], op=Alu.mult)
                mcomb = chbig.tile([128, NCOLS], F32, tag="mcomb")
                nc.vector.tensor_tensor(out=mcomb[:], in0=x[:, 0].bitcast(F32).rearrange("p b s -> p (b s)"),
                                        in1=x[:, 1].bitcast(F32).rearrange("p b s -> p (b s)"), op=Alu.max)
                nc.vector.tensor_tensor(out=mcomb[:], in0=mcomb[:],
                                        in1=x[:, 2].bitcast(F32).rearrange("p b s -> p (b s)"), op=Alu.max)
                nc.vector.tensor_tensor(out=mcomb[:], in0=mcomb[:],
                                        in1=x[:, 3].bitcast(F32).rearrange("p b s -> p (b s)"), op=Alu.max)
                mxall = chbig.tile([128, NCOLS], F32, tag="mxall")
                nc.gpsimd.partition_all_reduce(mxall[:], mcomb[:], channels=128,
                                               reduce_op=bass_isa.ReduceOp.max)
                st["mxall"] = mxall

            def epi_C(st):  # channel avg via ones-matmul
                sps = psp.tile([1, NCOLS], F32, tag="sp")
                for mt in range(MT):
                    nc.tensor.matmul(sps[:], avg_ones_r[:], st["x"][:, mt], start=(mt == 0), stop=(mt == MT - 1))
                avg_sb = chsm.tile([1, NCOLS], F32, tag="avg_sb")
                nc.vector.tensor_copy(out=avg_sb, in_=sps[:])
                st["avg_sb"] = avg_sb

            def epi_D(st):  # transpose avg/max rows into [hw, (ch, b)] bf16
                catTa = chsm.tile([HWA, 2, NBC], BF16, tag="catTa")
                catTb = chsm.tile([HWB, 2, NBC], BF16, tag="catTb")
                for b in range(NBC):
                    for src_i, src in enumerate((st["avg_sb"], st["mxall"])):
                        tpa = psp.tile([HWA, 1], F32, tag="sp")
                        nc.tensor.transpose(tpa[:], src[0:1, b * HW: b * HW + HWA], ident[0:1, 0:1])
                        nc.vector.tensor_copy(out=catTa[:, src_i, b:b + 1], in_=tpa[:])
                        tpb = psp.tile([HWB, 1], F32, tag="sp")
                        nc.tensor.transpose(tpb[:], src[0:1, b * HW + HWA: (b + 1) * HW], ident[0:1, 0:1])
                        nc.vector.tensor_copy(out=catTb[:, src_i, b:b + 1], in_=tpb[:])
                st["catTa"], st["catTb"] = catTa, catTb

            def epi_E(st):  # 7x7 spatial conv (as matmul) + sigmoid
                saps = psp.tile([NBC, HW], F32, tag="sp")
                first = True
                for ch in range(2):
                    nc.tensor.matmul(saps[:], st["catTa"][:, ch, :], saWa[:, ch, :], start=first, stop=False)
                    first = False
                    nc.tensor.matmul(saps[:], st["catTb"][:, ch, :], saWb[:, ch, :], start=False,
                                     stop=(ch == 1))
                tt = chsm.tile([NBC, HW], F32, tag="satanh")
                nc.scalar.activation(out=tt[:], in_=saps[:], func=Act.Tanh, scale=0.5)
                sig2 = chsm.tile([NBC, HW], F32R, tag="sig2")
                nc.vector.tensor_scalar(out=sig2[:], in0=tt[:], scalar1=0.5, scalar2=0.5,
                                        op0=Alu.mult, op1=Alu.add)
                st["sig2"] = sig2

            def epi_F(st):  # sigmoid broadcast + fused weighted spatial sum
                g = st["g"]
                sgps = psp.tile([128, NBC, HWP], F32, tag="sp")
                for b in range(NBC):
                    nc.tensor.matmul(sgps[:, b, 0:HW], sel1_r[:, b, :], st["sig2"][:],
                                     start=True, stop=True)
                for mt in range(MT):
                    for b in range(NBC):
                        ttro = chbig.tile([128, HW], F32, tag="ttro")
                        nc.vector.scalar_tensor_tensor(
                            out=ttro[:], in0=st["x"][:, mt, b].bitcast(F32), scalar=1.0,
                            in1=sgps[:, b, 0:HW], op0=Alu.mult, op1=Alu.mult,
                            accum_out=xsumT[:, mt, g * NBC + b: g * NBC + b + 1])

            states = {}
            full = STAGE >= 2
            a2T_g = None
            for gi in range(NCH + 2):
                cur = None
                if gi < NCH:
                    cur = {"g": gi}
                    states[gi] = cur
                    emit_dma(cur)
                p1 = states.get(gi - 1) if full else None
                p2 = states.get(gi - 2) if full else None
                for mt in range(MT):
                    if cur is not None:
                        emit_conv_mt(cur, mt)
                    if mt == 0:
                        if p2: epi_E(p2)
                        if p1: epi_A(p1)
                    elif mt == 1:
                        if p2: epi_F(p2)
                        if p1: epi_B(p1)
                    elif mt == 2:
                        if gi + 2 < NCH:
                            emit_feat(gi + 2)
                        if p1: epi_C(p1)
                    elif mt == 3:
                        if p1: epi_D(p1)
                if gi == 0 and full:
                    emit_sa_weights()
                if gi == 1 and full:
                    emit_tail_weights()
                    a2T_g = emit_tail_tables()
                if gi - 2 in states:
                    del states[gi - 2]
            if full and a2T_g is None:
                emit_tail_weights()
                a2T_g = emit_tail_tables()

            # ---------------- ACA + masks + l2norm ----------------
            if not full:
                out_stub = sg.tile([BL, C_OUT], F32)
                nc.vector.memset(out_stub[:], 0.0)
                nc.sync.dma_start(out=out_e[:], in_=out_stub)
                return nc

            xsr = sg.tile([128, MT, BL], F32R)
            nc.vector.tensor_copy(out=xsr, in_=xsumT)

            mT = sg.tile([128, MT, BL], F32R)
            for mt in range(MT):
                fps = psp.tile([128, BL], F32, tag="sp")
                for jt in range(8):
                    rhs = xsr[:, jt, :] if jt < 4 else a2T_g[:, jt - 4, :]
                    nc.tensor.matmul(fps[:], fc1wT[:, jt, mt * 128:(mt + 1) * 128], rhs,
                                     start=(jt == 0), stop=(jt == 7))
                nc.scalar.activation(out=mT[:, mt, :], in_=fps[:], func=Act.Relu,
                                     bias=vcols[:, IV["fc1_b"], mt:mt + 1])

            mm_sb = sg.tile([128, MT, BL], F32)
            mmt = sg.tile([128, MT, BL], F32)
            fc2b_half = sg.tile([128, MT], F32)
            nc.vector.tensor_scalar(out=fc2b_half[:], in0=vcols[:, IV["fc2_b"], :], scalar1=0.5,
                                    scalar2=None, op0=Alu.mult)
            for mt in range(MT):
                fps = psp.tile([128, BL], F32, tag="sp")
                for jt in range(4):
                    nc.tensor.matmul(fps[:], fc2wT[:, jt, mt * 128:(mt + 1) * 128], mT[:, jt, :],
                                     start=(jt == 0), stop=(jt == 3))
                nc.scalar.activation(out=mmt[:, mt, :], in_=fps[:], func=Act.Tanh,
                                     bias=fc2b_half[:, mt:mt + 1], scale=0.5)
                nc.vector.tensor_scalar(out=mm_sb[:, mt, :], in0=mmt[:, mt, :], scalar1=0.5,
                                        scalar2=0.5, op0=Alu.mult, op1=Alu.add)

            y = sg.tile([128, MT, BL], F32)
            nc.vector.scalar_tensor_tensor(out=y[:], in0=xsumT[:], scalar=w2_bc[:], in1=mm_sb[:],
                                           op0=Alu.mult, op1=Alu.mult)
            nc.vector.tensor_tensor(out=y[:], in0=y[:], in1=masksT[:], op=Alu.mult)

            ysq = sg.tile([128, MT, BL], F32R)
            nc.vector.tensor_tensor(out=ysq[:], in0=y[:], in1=y[:], op=Alu.mult)
            l2ps = psp.tile([1, BL], F32, tag="sp")
            for mt in range(MT):
                nc.tensor.matmul(l2ps[:], one_col_r[:], ysq[:, mt, :], start=(mt == 0), stop=(mt == MT - 1))
            s2 = sg.tile([1, BL], F32)
            nc.vector.tensor_copy(out=s2, in_=l2ps[:])
            snorm = sg.tile([1, BL], F32)
            nc.scalar.activation(out=snorm[:], in_=s2[:], func=Act.Sqrt)
            rinv = sg.tile([1, BL], F32)
            nc.vector.reciprocal(out=rinv[:], in_=snorm[:])
            rbc = sg.tile([128, BL], F32)
            nc.gpsimd.partition_broadcast(out_ap=rbc[:], in_ap=rinv[:])

            yn = sg.tile([128, MT, BL], F32)
            for mt in range(MT):
                nc.vector.tensor_tensor(out=yn[:, mt, :], in0=y[:, mt, :], in1=rbc[:], op=Alu.mult)

            out_sb = sg.tile([BL, MT, 128], F32)
            for mt in range(MT):
                ops_ = psp.tile([BL, 128], F32, tag="sp")
                nc.tensor.transpose(ops_[:], yn[:, mt, :], ident[:])
                nc.vector.tensor_copy(out=out_sb[:, mt, :], in_=ops_[:])
            nc.sync.dma_start(out=out_e[:], in_=out_sb)

    return nc


def _build_sa_toeplitz(sa_w):
    """sa_w: [1, 2, 7, 7] -> T[ch, hw_src, hw_dst] for 'same' 7x7 cross-correlation."""
    T = np.zeros((2, HW, HW), np.float32)
    for ch in range(2):
        for dy in range(7):
            for dx in range(7):
                v = float(sa_w[0, ch, dy, dx])
                if v == 0.0:
                    continue
                ylo, yhi = max(0, 3 - dy), min(14, 17 - dy)
                xlo, xhi = max(0, 3 - dx), min(14, 17 - dx)
                for yp in range(ylo, yhi):
                    ys = yp + dy - 3
                    row = ys * 14
                    dst = yp * 14
                    for xp in range(xlo, xhi):
                        T[ch, row + (xp + dx - 3), dst + xp] = v
    return T


_CACHE = {}


def _get_nc():
    if "nc" not in _CACHE:
        nc = build_nc()
        nc.finalize()
        _CACHE["nc"] = nc
    return _CACHE["nc"]


def _prep_maps(inputs):
    f32 = np.float32
    feat = np.ascontiguousarray(np.asarray(inputs["feat"], f32).reshape(B, C_IN, HW))
    c = np.ascontiguousarray(np.asarray(inputs["c"]).astype(np.int32))

    vcols = np.zeros((128, len(VECS), MT), f32)
    for name in VECS:
        vcols[:, IV[name], :] = np.asarray(inputs[name], f32).reshape(MT, 128).T

    shared = {
        "conv1wT": np.ascontiguousarray(np.asarray(inputs["conv1_w"], f32).T),
        "attr_embT": np.ascontiguousarray(np.asarray(inputs["attr_emb"], f32).T),
        "t1wT": np.ascontiguousarray(np.asarray(inputs["t1_w"], f32).T),
        "t2wT": np.ascontiguousarray(np.asarray(inputs["t2_w"], f32).T),
        "fc1wT": np.ascontiguousarray(np.asarray(inputs["fc1_w"], f32).T),
        "fc2wT": np.ascontiguousarray(np.asarray(inputs["fc2_w"], f32).T),
        "t1_b": np.asarray(inputs["t1_b"], f32),
        "t2_b": np.asarray(inputs["t2_b"], f32),
        "vcols": vcols,
        "masks_w": np.asarray(inputs["masks_w"], f32),
        "sa_toep": _build_sa_toeplitz(np.asarray(inputs["sa_w"], f32)),
        "w": np.asarray(inputs["w"], f32),
        "w1": np.asarray(inputs["w1"], f32),
        "w2": np.asarray(inputs["w2"], f32),
    }
    in_maps = []
    for i in range(N_CORES):
        m = dict(shared)
        m["feat"] = feat[i * BL:(i + 1) * BL]
        m["c"] = c[i * BL:(i + 1) * BL]
        in_maps.append(m)
    return in_maps


def run_spmd(inputs, trace=False):
    nc = _get_nc()
    in_maps = _prep_maps(inputs)
    res = run_bass_kernel_spmd(nc, in_maps, core_ids=list(range(N_CORES)), trace=trace)
    out = np.concatenate([res.results[i]["out"] for i in range(N_CORES)], axis=0)
    return out.astype(np.float32), res


def kernel(**inputs):
    out, _ = run_spmd(inputs, trace=False)
    return out
